# revision 18
# baseline (speedup 1.0000x reference)
"""DeepSpeedAttention (B=2, S=2048, H=4096, 32 heads) on 8 Trainium2 cores.

Sharding: tensor-parallel across heads. Each core computes QKV for its 4
heads (column shard of attn_qkvw), full attention for those heads, and a
partial output projection (row shard of attn_ow). The 8 partial outputs are
summed on the host (host-side all-reduce) and the output bias is added.

Device kernel layout (per core):
  xT   [4096 H, 4096 tok]   bf16  (x transposed host-side; replicated)
  wq/wk[4096 H, 512]        bf16  (Q/K column shards)
  wv   [4096 H, 512]        bf16
  wo   [512, 4096]          bf16  (row shard of attn_ow)
  out  [4096 tok, 4096]     f32   (partial result, summed on host)

Phase A: QKV projection (startup DMAs chunked so matmuls start early).
  qT,kT computed transposed ([col, tok]) with the weight stationary; v
  natural ([tok, col]). Biases fused into the DVE PSUM eviction
  (tensor_scalar_add for q/k, rank-1 ones-matmul for v). q/k/v staged to
  DRAM; the first two attention head tile-sets are prefetched from inside
  phase A's instruction stream so the A->B seam has no DMA wait.
Phase B: attention per (batch, local head), software-pipelined kj units.
  A kj unit = two 128-key score matmuls into one 2-bank PSUM tile + one
  1024-wide exp on ACT (softmax scale folded in). PV matmuls for unit u are
  emitted after the scores of unit u+1, so the ACT exp latency is hidden
  behind TensorE work. ctx accumulates UNNORMALIZED in PSUM and is evicted
  with a plain copy (no dependency on the softmax denominator), then
  normalized in-place in SBUF once the denominator chain (bf16 pairwise
  adds + tree on DVE, gpsimd partition_all_reduce, reciprocal) completes
  off the critical path.
Phase C: output projection. C tiles are interleaved one-per-kj-unit into
  batch 1's phase B stream as TensorE filler (batch 0's ctxT is complete by
  then); the rest drain at the end. PSUM evictions alternate ACT/DVE.
"""

import os
import numpy as np
import ml_dtypes
from contextlib import ExitStack

try:
    import jax
    jax.config.update(
        "jax_compilation_cache_dir", os.path.expanduser("~/.bass_jax_cache"))
    jax.config.update("jax_persistent_cache_min_compile_time_secs", 10.0)
    jax.config.update("jax_persistent_cache_min_entry_size_bytes", 0)
except Exception:
    pass

import concourse.bass as bass
from concourse import bass_isa
import concourse.tile as tile
from concourse import bacc, mybir
from concourse.bass_utils import run_bass_kernel_spmd

BF16 = mybir.dt.bfloat16
F32 = mybir.dt.float32
AF = mybir.ActivationFunctionType

H = 4096          # hidden
TOK = 4096        # B*S tokens
S = 2048          # seq len per batch
NB = 2            # batches
HL = 4            # heads per core
HD = 128          # head dim
COLS = HL * HD    # per-core hidden shard (512)
NCORES = 8
KT = H // 128     # 32 contraction tiles for the projections
NKT = S // 128    # 16 k tiles per batch
SCALE = 1.0 / float(np.sqrt(HD))


def build_nc(phases: str = "ABC"):
    nc = bacc.Bacc("TRN2", target_bir_lowering=False, debug=False)

    xT = nc.dram_tensor("xT", [H, TOK], BF16, kind="ExternalInput").ap()
    wq = nc.dram_tensor("wq", [H, COLS], BF16, kind="ExternalInput").ap()
    wk = nc.dram_tensor("wk", [H, COLS], BF16, kind="ExternalInput").ap()
    wv = nc.dram_tensor("wv", [H, COLS], BF16, kind="ExternalInput").ap()
    bq = nc.dram_tensor("bq", [1, COLS], F32, kind="ExternalInput").ap()
    bk = nc.dram_tensor("bk", [1, COLS], F32, kind="ExternalInput").ap()
    bv = nc.dram_tensor("bv", [1, COLS], BF16, kind="ExternalInput").ap()
    wo = nc.dram_tensor("wo", [COLS, H], BF16, kind="ExternalInput").ap()
    out = nc.dram_tensor("out", [TOK, H], F32, kind="ExternalOutput").ap()

    with tile.TileContext(nc) as tc, ExitStack() as ctx:
        dram = ctx.enter_context(tc.tile_pool(name="dram", bufs=1, space="DRAM"))
        # per-batch staging tiles: batch-0 readers (prefetched from inside
        # phase A) must not inherit a whole-tile dependency on batch-1 writes
        qT_ds = [dram.tile([COLS, S], BF16, name=f"qT_d{b}") for b in range(NB)]
        kT_ds = [dram.tile([COLS, S], BF16, name=f"kT_d{b}") for b in range(NB)]
        v_ds = [dram.tile([S, COLS], BF16, name=f"v_d{b}") for b in range(NB)]

        const = ctx.enter_context(tc.tile_pool(name="const", bufs=1))
        ones_bf = const.tile([1, 512], BF16)
        nc.vector.memset(ones_bf[:], 1.0)
        # warm up the gpsimd engine (library load takes ~3.5us on first
        # use) long before phase B's first softmax-denominator reduce
        gpw = const.tile([128, 1], F32)
        nc.vector.memset(gpw[:], 1.0)
        nc.gpsimd.partition_all_reduce(
            gpw[:], gpw[:], channels=128, reduce_op=bass_isa.ReduceOp.add)
        # per-partition layout [col-within-tile, col-tile] for tensor_scalar
        bq_sb = const.tile([128, HL], F32)
        nc.sync.dma_start(bq_sb[:], bq.rearrange("o (ct p) -> p (o ct)", p=128))
        bk_sb = const.tile([128, HL], F32)
        nc.sync.dma_start(bk_sb[:], bk.rearrange("o (ct p) -> p (o ct)", p=128))
        bv_sb = const.tile([1, COLS], BF16)
        nc.sync.dma_start(bv_sb[:], bv)

        # Attention input tiles for the first two (batch 0) head pairs are
        # allocated up front so their DMAs can be emitted from inside phase
        # A's stream (prefetch across the A->B seam).
        bqk0 = ctx.enter_context(tc.tile_pool(name="bqk0", bufs=2))

        pair_tiles = {}

        def ensure_pair_tiles(pi, pool):
            if pi in pair_tiles:
                return
            b, hl = divmod(pi, HL)
            r0 = hl * 128
            # Pairs 0/1 (dedicated bqk0 space, pure data deps) are issued
            # from the ACT HWDGE queue: the sync engine's in-order stream is
            # saturated by phase A's eviction triggers and its queues only
            # complete these reads after all of phase A. Later pairs reuse
            # released phase-A SBUF, so they stay on the sync queue where
            # the WAR ordering is naturally enforced.
            eng = nc.scalar if pi < 2 else nc.sync
            qh = pool.tile([128, S], BF16, tag="qh", name=f"qh{pi}")
            eng.dma_start(qh[:], qT_ds[b][r0:r0 + 128, :])
            kh = pool.tile([128, S], BF16, tag="kh", name=f"kh{pi}")
            eng.dma_start(kh[:], kT_ds[b][r0:r0 + 128, :])
            vh = pool.tile([128, NKT, 128], BF16, tag="vh", name=f"vh{pi}")
            eng.dma_start(
                vh[:],
                v_ds[b][:, r0:r0 + 128].rearrange("(i p) d -> p i d", p=128),
            )
            pair_tiles[pi] = (qh, kh, vh)

        # ---------------- Phase A: QKV projection ----------------
        if "A" in phases:
         with tc.tile_pool(name="aw", bufs=1) as awp, \
             tc.tile_pool(name="ax", bufs=2) as axp, \
             tc.tile_pool(name="ast", bufs=6) as astp, \
             tc.tile_pool(name="aps", bufs=4, space="PSUM") as apsp:
            # wq + first x chunk first, split into 8-ktile chunks so the
            # first matmuls start after ~2MB of DMA instead of ~8.4MB.
            wq_sb = awp.tile([128, KT, COLS], BF16)
            wqr = wq.rearrange("(kt p) c -> p kt c", p=128)
            x0_sb = axp.tile([128, KT, 512], BF16, tag="x")
            x0r = xT[:, 0:512].rearrange("(kt p) t -> p kt t", p=128)
            chunks = [(0, 2), (2, 4)] + [(k, k + 4) for k in range(4, 32, 4)]
            for lo, hi in chunks:
                ks = slice(lo, hi)
                nc.sync.dma_start(wq_sb[:, ks, :], wqr[:, ks, :])
                nc.sync.dma_start(x0_sb[:, ks, :], x0r[:, ks, :])
            # wk in 8-ktile pieces: chunk 0's k groups run kt-outer and
            # consume these as they land (startup is DMA-bandwidth-paced)
            wk_sb = awp.tile([128, KT, COLS], BF16)
            wkr = wk.rearrange("(kt p) c -> p kt c", p=128)
            for k0 in range(0, KT, 8):
                nc.sync.dma_start(wk_sb[:, k0:k0 + 8, :], wkr[:, k0:k0 + 8, :])
            wv_sb = awp.tile([128, KT, COLS], BF16)
            nc.sync.dma_start(wv_sb[:], wv.rearrange("(kt p) c -> p kt c", p=128))

            for tck in range(TOK // 512):
                t0 = tck * 512
                if tck == 0:
                    x_sb = x0_sb
                else:
                    x_sb = axp.tile([128, KT, 512], BF16, tag="x")
                    nc.sync.dma_start(
                        x_sb[:],
                        xT[:, t0:t0 + 512].rearrange("(kt p) t -> p kt t", p=128),
                    )
                bb = tck // 4       # which batch these tokens belong to
                tl = t0 - bb * S    # token offset within the batch
                # qT / kT: [col-tile 128, tok 512], weight stationary.
                # Chunk 0 runs kt-outer (4 col groups per arriving DMA
                # chunk) so the DMA-paced startup never starves TensorE;
                # later chunks have DMA far ahead and run ct-outer.
                for half in range(2):
                    cts = range(half * 4, half * 4 + 4)
                    w_sb = wq_sb if half == 0 else wk_sb
                    b_sb = bq_sb if half == 0 else bk_sb
                    dst = qT_ds[bb] if half == 0 else kT_ds[bb]
                    pss = {}
                    for ct in cts:
                        pss[ct] = apsp.tile([128, 512], F32, tag="qk",
                                            name=f"apsqk{tck}_{ct}")
                    if tck == 0:
                        for kt in range(KT):
                            for ct in cts:
                                c0 = (ct % 4) * 128
                                nc.tensor.matmul(
                                    pss[ct][:], w_sb[:, kt, c0:c0 + 128],
                                    x_sb[:, kt, :],
                                    start=(kt == 0), stop=(kt == KT - 1),
                                )
                    else:
                        for ct in cts:
                            c0 = (ct % 4) * 128
                            for kt in range(KT):
                                nc.tensor.matmul(
                                    pss[ct][:], w_sb[:, kt, c0:c0 + 128],
                                    x_sb[:, kt, :],
                                    start=(kt == 0), stop=(kt == KT - 1),
                                )
                    for ct in cts:
                        c0 = (ct % 4) * 128
                        # eviction on DVE (idle in phase A) w/ fused bias add
                        st = astp.tile([128, 512], BF16, tag="qk_st")
                        nc.vector.tensor_scalar_add(
                            st[:], pss[ct][:], b_sb[:, ct % 4:ct % 4 + 1])
                        nc.sync.dma_start(dst[c0:c0 + 128, tl:tl + 512], st[:])
                # v: [tok-tile 128, col 512], x stationary
                for tt in range(4):
                    ps = apsp.tile([128, 512], F32, tag="v")
                    for kt in range(KT):
                        nc.tensor.matmul(
                            ps[:], x_sb[:, kt, tt * 128:(tt + 1) * 128],
                            wv_sb[:, kt, :],
                            start=(kt == 0), stop=False,
                        )
                    # bias: out[tok, col] += ones[tok] x b[col]
                    nc.tensor.matmul(
                        ps[:], ones_bf[:, 0:128], bv_sb[:],
                        start=False, stop=True,
                    )
                    st = astp.tile([128, 512], BF16, tag="v_st")
                    # ACT is idle all of phase A; splitting evictions
                    # ACT/DVE also drains the A tail faster (B's first
                    # score PSUM banks wait on A's last evictions)
                    nc.scalar.copy(st[:], ps[:])
                    nc.sync.dma_start(
                        v_ds[bb][tl + tt * 128:tl + (tt + 1) * 128, :], st[:])
                if tck == 3 and os.environ.get("PREFETCH", "1") == "1":
                    # batch-0 q/k/v fully staged: prefetch the first two
                    # head pairs now so phase B starts without a DMA wait.
                    ensure_pair_tiles(0, bqk0)
                    ensure_pair_tiles(1, bqk0)

        # ---------------- Phase B + C ----------------
        # ctxT survives phase B into phase C: [d, head, tok].
        ctxp = ctx.enter_context(tc.tile_pool(name="ctxp", bufs=1))
        ctxT = ctxp.tile([128, HL, TOK], BF16)

        # wo_sb's DMA is emitted from inside the B loop (see below): DMA
        # completion semaphores are shared across queues, so any matmul
        # emitted after a dma_start transitively waits on it -- emitting the
        # 4MB wo load at B's head would stall B's first PV group on it.
        cwp = ctx.enter_context(tc.tile_pool(name="cw", bufs=1))
        wo_sb = cwp.tile([128, HL, H], BF16)

        with tc.tile_pool(name="bqk", bufs=6) as bqkp, \
             tc.tile_pool(name="bpr", bufs=4) as bprp, \
             tc.tile_pool(name="bt8", bufs=2) as bt8p, \
             tc.tile_pool(name="bst", bufs=1) as bstp, \
             tc.tile_pool(name="bs2", bufs=2) as bst2p, \
             tc.tile_pool(name="bsc", bufs=2, space="PSUM") as bscp, \
             tc.tile_pool(name="bcx", bufs=2, space="PSUM") as bcxp, \
             tc.tile_pool(name="cst", bufs=4) as cstp, \
             tc.tile_pool(name="cps", bufs=2, space="PSUM") as cpsp:

            c_state = {"n": 0, "ready": 0, "caps": [(0, 0)]}
            PV_DEPTH = int(os.environ.get("PV_DEPTH", "2"))
            C_LAG = int(os.environ.get("C_LAG", "6"))
            TAIL2_LAG = int(os.environ.get("TAIL2_LAG", "3"))

            def emit_c_tile():
                # next output-projection tile, token-major within batch
                i = c_state["n"]
                c_state["n"] += 1
                ot, ncol = divmod(i, H // 512)
                t0 = ot * 128
                n0 = ncol * 512
                ps = cpsp.tile([128, 512], F32, tag="op", name=f"ops{i}")
                for hl in range(HL):
                    nc.tensor.matmul(
                        ps[:], ctxT[:, hl, t0:t0 + 128],
                        wo_sb[:, hl, n0:n0 + 512],
                        start=(hl == 0), stop=(hl == HL - 1),
                    )
                st = cstp.tile([128, 512], F32, tag="ost", name=f"ost{i}")
                if i % 2 == 0:
                    nc.scalar.copy(st[:], ps[:])
                else:
                    nc.vector.tensor_copy(st[:], ps[:])
                nc.sync.dma_start(out[t0:t0 + 128, n0:n0 + 512], st[:])

            # flat kj-unit schedule, qc-major within each batch: all four
            # heads finish a given qc block together, so output-projection
            # tiles for those tokens become C filler one qc later -- during
            # most of batch 0's otherwise ACT-bound window.
            units = [(b * HL + hl, qc, kj)
                     for b in range(NB)
                     for qc in range(S // 512)
                     for hl in range(HL)
                     for kj in range(NKT // 2)]

            state = {}   # per live qc: (cps, tmp8)
            pending = []  # units waiting for their PV emission (depth PV_DEPTH)

            def emit_pv(u):
                pi, qc, kj, probs_u = u
                _, _, vh = pair_tiles[pi]
                cps, _ = state[(pi, qc)]
                for ui in range(2):
                    ki = 2 * kj + ui
                    nc.tensor.matmul(
                        cps[:], vh[:, ki, :], probs_u[:, ui, :],
                        start=(ki == 0), stop=(ki == NKT - 1),
                    )

            deferred = []  # (emit_unit, lsb, dst, bump) awaiting recip+mul

            def emit_qc_tail(u, un):
                # ctx eviction (unnormalized) + softmax denominator adds +
                # gpsimd partition reduce. The reciprocal + normalize are
                # DEFERRED a few units (emit_qc_tail2): a reciprocal whose
                # gpsimd input isn't ready yet head-of-line blocks the DVE
                # FIFO for ~3.5us, which backlogs tmp8 adds/C evictions and
                # stalls the PE queue behind C-tile matmuls.
                pi, qc, kj, _ = u
                b, hl = divmod(pi, HL)
                s0 = b * S
                q0 = qc * 512
                cps, tmp8 = state.pop((pi, qc))
                dst = ctxT[:, hl, s0 + q0:s0 + q0 + 512]
                # evict on ACT: DVE's queue (pairwise adds + C evictions)
                # backlogs this copy and stalls the PV ring by ~1.3us per qc
                nc.scalar.copy(dst, cps[:])
                t4 = bstp.tile([128, 4, 512], BF16, tag="t4")
                nc.vector.tensor_add(
                    t4[:], tmp8[:, 0:8:2, :], tmp8[:, 1:8:2, :])
                t2 = bstp.tile([128, 2, 512], BF16, tag="t2")
                nc.vector.tensor_add(
                    t2[:], t4[:, 0:4:2, :], t4[:, 1:4:2, :])
                acc = bstp.tile([128, 512], F32, tag="acc")
                nc.vector.tensor_add(acc[:], t2[:, 0, :], t2[:, 1, :])
                lsb = bst2p.tile([128, 512], F32, tag="lsb",
                                 name=f"lsb{pi}_{qc}")
                nc.gpsimd.partition_all_reduce(
                    lsb[:], acc[:], channels=128,
                    reduce_op=bass_isa.ReduceOp.add)
                deferred.append((un, lsb, dst, pi % HL == HL - 1))

            def emit_qc_tail2(lsb, dst, bump, un):
                rec = bst2p.tile([128, 512], F32, tag="rec")
                nc.vector.reciprocal_approx_fast(out=rec[:], in_=lsb[:])
                nc.vector.tensor_mul(dst, dst, rec[:])
                if bump:
                    # all 4 heads of this qc normalized (in emission order):
                    # its 32 C tiles become releasable C_LAG units later
                    c_state["ready"] += 32
                    c_state["caps"].append((un + C_LAG, c_state["ready"]))

            if "B" in phases:
                npairs = NB * HL
                for un, (pi, qc, kj) in enumerate(units):
                    b, hl = divmod(pi, HL)
                    s0 = b * S
                    q0 = qc * 512
                    if un == 0:
                        for pn in range(HL):
                            ensure_pair_tiles(pn, bqk0 if pn < 2 else bqkp)
                    elif un == 32:
                        # wo load is emitted here (not at B's head): DMA
                        # completion semaphores are shared, so everything
                        # emitted after it would transitively wait for it
                        nc.sync.dma_start(
                            wo_sb[:],
                            wo.rearrange("(hl p) n -> p hl n", p=128))
                    elif un == 64:
                        ensure_pair_tiles(HL, bqkp)
                        ensure_pair_tiles(HL + 1, bqkp)
                    elif un == 96:
                        ensure_pair_tiles(HL + 2, bqkp)
                        ensure_pair_tiles(HL + 3, bqkp)
                    qh, kh, _ = pair_tiles[pi]
                    if kj == 0:
                        cps = bcxp.tile([128, 512], F32, tag="ctx",
                                        name=f"cps{pi}_{qc}")
                        tmp8 = bt8p.tile([128, NKT // 2, 512], BF16,
                                         tag="tmp8", bufs=1,
                                         name=f"tmp8{pi}_{qc}")
                        state[(pi, qc)] = (cps, tmp8)
                    # scores for this unit: two 128-key tiles, one 2-bank
                    # PSUM tile so the exp runs 1024 wide
                    sps = bscp.tile([128, 2, 512], F32, tag="sc",
                                    name=f"sps{un}")
                    for ui in range(2):
                        ki = 2 * kj + ui
                        nc.tensor.matmul(
                            sps[:, ui, :],
                            kh[:, ki * 128:(ki + 1) * 128],
                            qh[:, q0:q0 + 512], start=True, stop=True,
                        )
                    # PV of an earlier unit lands here, after this unit's
                    # score matmuls. Depth-2 queue gives the ACT exp two
                    # units of TensorE work (~2.5us) of slack, so PV never
                    # blocks on a just-in-time exp completion.
                    while len(pending) >= PV_DEPTH:
                        u = pending.pop(0)
                        emit_pv(u)
                        if u[2] == NKT // 2 - 1:
                            emit_qc_tail(u, un)
                    # deferred recip+mul once the gpsimd reduce has had
                    # TAIL2_LAG units (~3us) to complete
                    while deferred and deferred[0][0] <= un - TAIL2_LAG:
                        _, lsb, dst, bump = deferred.pop(0)
                        emit_qc_tail2(lsb, dst, bump, un)
                    probs_u = bprp.tile([128, 2, 512], BF16, tag="probs",
                                        name=f"probs{un}")
                    nc.scalar.activation(probs_u[:], sps[:], AF.Exp,
                                         scale=SCALE)
                    _, tmp8 = state[(pi, qc)]
                    nc.vector.tensor_add(
                        tmp8[:, kj, :], probs_u[:, 0, :], probs_u[:, 1, :])
                    pending.append((pi, qc, kj, probs_u))
                    # C filler: one output tile per unit once its ctxT
                    # sources are done. During the last pair of each batch,
                    # token blocks of already-finished qc become available.
                    if "C" in phases:
                        # release at most one tile per unit, gated on the
                        # source qc's tails having been EMITTED (emission-
                        # order safety with the deeper PV queue) plus a
                        # C_LAG-unit cooldown for the normalize chain
                        cap = 0
                        for ua, cv in c_state["caps"]:
                            if un >= ua:
                                cap = max(cap, cv)
                        if c_state["n"] < cap:
                            emit_c_tile()
                for u in pending:
                    emit_pv(u)
                    if u[2] == NKT // 2 - 1:
                        emit_qc_tail(u, len(units))
                pending = []
                for ud, lsb, dst, bump in deferred:
                    emit_qc_tail2(lsb, dst, bump, len(units))
                deferred = []

            # ---------------- Phase C drain ----------------
            if "C" in phases:
                while c_state["n"] < TOK // 128 * (H // 512):
                    emit_c_tile()

    nc.compile()
    return nc


_NC = None


def _get_nc():
    global _NC
    if _NC is None:
        _NC = build_nc()
    return _NC


def _shard_inputs(x, attn_qkvw, attn_qkvb, attn_ow):
    bf = ml_dtypes.bfloat16
    x = np.asarray(x, dtype=np.float32)
    w = np.asarray(attn_qkvw, dtype=np.float32)
    b = np.asarray(attn_qkvb, dtype=np.float32)
    wo = np.asarray(attn_ow, dtype=np.float32)

    xT = np.ascontiguousarray(x.reshape(TOK, H).T).astype(bf)
    w4 = w.reshape(H, 3, 32, HD)
    b4 = b.reshape(3, 32, HD)
    in_maps = []
    for c in range(NCORES):
        hs = slice(c * HL, (c + 1) * HL)
        in_maps.append({
            "xT": xT,
            "wq": np.ascontiguousarray(w4[:, 0, hs, :].reshape(H, COLS)).astype(bf),
            "wk": np.ascontiguousarray(w4[:, 1, hs, :].reshape(H, COLS)).astype(bf),
            "wv": np.ascontiguousarray(w4[:, 2, hs, :].reshape(H, COLS)).astype(bf),
            "bq": np.ascontiguousarray(b4[0, hs, :].reshape(1, COLS)),
            "bk": np.ascontiguousarray(b4[1, hs, :].reshape(1, COLS)),
            "bv": b4[2, hs, :].reshape(1, COLS).astype(bf),
            "wo": np.ascontiguousarray(
                wo[c * COLS:(c + 1) * COLS, :]).astype(bf),
        })
    return in_maps


def kernel(x, attn_qkvw, attn_qkvb, attn_ow, attn_ob):
    import time as _time
    nc = _get_nc()
    in_maps = _shard_inputs(x, attn_qkvw, attn_qkvb, attn_ow)
    res = None
    for attempt in range(3):
        try:
            res = run_bass_kernel_spmd(nc, in_maps, core_ids=list(range(NCORES)))
            break
        except Exception:
            # transient NRT_EXEC_UNIT_UNRECOVERABLE has been observed on a
            # first dispatch; rebuild and retry once before giving up
            if attempt == 2:
                raise
            _time.sleep(2)
            global _NC
            _NC = None
            nc = _get_nc()
    acc = res.results[0]["out"]
    for c in range(1, NCORES):
        acc = acc + res.results[c]["out"]
    acc = acc + np.asarray(attn_ob, dtype=np.float32)[None, :]
    return acc.reshape(NB, S, H)



# revision 20
# speedup vs baseline: 1.0071x; 1.0071x over previous
"""DeepSpeedAttention (B=2, S=2048, H=4096, 32 heads) on 8 Trainium2 cores.

Sharding: tensor-parallel across heads. Each core computes QKV for its 4
heads (column shard of attn_qkvw), full attention for those heads, and a
partial output projection (row shard of attn_ow). The 8 partial outputs are
summed on the host (host-side all-reduce) and the output bias is added.

Device kernel layout (per core):
  xT   [4096 H, 4096 tok]   bf16  (x transposed host-side; replicated)
  wq/wk[4096 H, 512]        bf16  (Q/K column shards)
  wv   [4096 H, 512]        bf16
  wo   [512, 4096]          bf16  (row shard of attn_ow)
  out  [4096 tok, 4096]     f32   (partial result, summed on host)

Phase A: QKV projection (startup DMAs chunked so matmuls start early).
  qT,kT computed transposed ([col, tok]) with the weight stationary; v
  natural ([tok, col]). Biases fused into the DVE PSUM eviction
  (tensor_scalar_add for q/k, rank-1 ones-matmul for v). q/k/v staged to
  DRAM; the first two attention head tile-sets are prefetched from inside
  phase A's instruction stream so the A->B seam has no DMA wait.
Phase B: attention per (batch, local head), software-pipelined kj units.
  A kj unit = two 128-key score matmuls into one 2-bank PSUM tile + one
  1024-wide exp on ACT (softmax scale folded in). PV matmuls for unit u are
  emitted after the scores of unit u+1, so the ACT exp latency is hidden
  behind TensorE work. ctx accumulates UNNORMALIZED in PSUM and is evicted
  with a plain copy (no dependency on the softmax denominator), then
  normalized in-place in SBUF once the denominator chain (bf16 pairwise
  adds + tree on DVE, gpsimd partition_all_reduce, reciprocal) completes
  off the critical path.
Phase C: output projection. C tiles are interleaved one-per-kj-unit into
  batch 1's phase B stream as TensorE filler (batch 0's ctxT is complete by
  then); the rest drain at the end. PSUM evictions alternate ACT/DVE.
"""

import os
import numpy as np
import ml_dtypes
from contextlib import ExitStack

try:
    import jax
    jax.config.update(
        "jax_compilation_cache_dir", os.path.expanduser("~/.bass_jax_cache"))
    jax.config.update("jax_persistent_cache_min_compile_time_secs", 10.0)
    jax.config.update("jax_persistent_cache_min_entry_size_bytes", 0)
except Exception:
    pass

import concourse.bass as bass
from concourse import bass_isa
import concourse.tile as tile
from concourse import bacc, mybir
from concourse.bass_utils import run_bass_kernel_spmd

BF16 = mybir.dt.bfloat16
F32 = mybir.dt.float32
AF = mybir.ActivationFunctionType

H = 4096          # hidden
TOK = 4096        # B*S tokens
S = 2048          # seq len per batch
NB = 2            # batches
HL = 4            # heads per core
HD = 128          # head dim
COLS = HL * HD    # per-core hidden shard (512)
NCORES = 8
KT = H // 128     # 32 contraction tiles for the projections
NKT = S // 128    # 16 k tiles per batch
SCALE = 1.0 / float(np.sqrt(HD))


def build_nc(phases: str = "ABC"):
    nc = bacc.Bacc("TRN2", target_bir_lowering=False, debug=False)

    xT = nc.dram_tensor("xT", [H, TOK], BF16, kind="ExternalInput").ap()
    wq = nc.dram_tensor("wq", [H, COLS], BF16, kind="ExternalInput").ap()
    wk = nc.dram_tensor("wk", [H, COLS], BF16, kind="ExternalInput").ap()
    wv = nc.dram_tensor("wv", [H, COLS], BF16, kind="ExternalInput").ap()
    bq = nc.dram_tensor("bq", [1, COLS], F32, kind="ExternalInput").ap()
    bk = nc.dram_tensor("bk", [1, COLS], F32, kind="ExternalInput").ap()
    bv = nc.dram_tensor("bv", [1, COLS], BF16, kind="ExternalInput").ap()
    wo = nc.dram_tensor("wo", [COLS, H], BF16, kind="ExternalInput").ap()
    out = nc.dram_tensor("out", [TOK, H], F32, kind="ExternalOutput").ap()

    with tile.TileContext(nc) as tc, ExitStack() as ctx:
        dram = ctx.enter_context(tc.tile_pool(name="dram", bufs=1, space="DRAM"))
        # per-batch staging tiles: batch-0 readers (prefetched from inside
        # phase A) must not inherit a whole-tile dependency on batch-1 writes
        qT_ds = [dram.tile([COLS, S], BF16, name=f"qT_d{b}") for b in range(NB)]
        kT_ds = [dram.tile([COLS, S], BF16, name=f"kT_d{b}") for b in range(NB)]
        v_ds = [dram.tile([S, COLS], BF16, name=f"v_d{b}") for b in range(NB)]

        const = ctx.enter_context(tc.tile_pool(name="const", bufs=1))
        ones_bf = const.tile([1, 512], BF16)
        nc.vector.memset(ones_bf[:], 1.0)
        # warm up the gpsimd engine (library load takes ~3.5us on first
        # use) long before phase B's first softmax-denominator reduce
        gpw = const.tile([128, 1], F32)
        nc.vector.memset(gpw[:], 1.0)
        nc.gpsimd.partition_all_reduce(
            gpw[:], gpw[:], channels=128, reduce_op=bass_isa.ReduceOp.add)
        # per-partition layout [col-within-tile, col-tile] for tensor_scalar
        bq_sb = const.tile([128, HL], F32)
        nc.sync.dma_start(bq_sb[:], bq.rearrange("o (ct p) -> p (o ct)", p=128))
        bk_sb = const.tile([128, HL], F32)
        nc.sync.dma_start(bk_sb[:], bk.rearrange("o (ct p) -> p (o ct)", p=128))
        bv_sb = const.tile([1, COLS], BF16)
        nc.sync.dma_start(bv_sb[:], bv)

        # Attention input tiles for the first two (batch 0) head pairs are
        # allocated up front so their DMAs can be emitted from inside phase
        # A's stream (prefetch across the A->B seam).
        bqk0 = ctx.enter_context(tc.tile_pool(name="bqk0", bufs=2))

        pair_tiles = {}

        def ensure_pair_tiles(pi, pool):
            if pi in pair_tiles:
                return
            b, hl = divmod(pi, HL)
            r0 = hl * 128
            # Pairs 0/1 (dedicated bqk0 space, pure data deps) are issued
            # from the ACT HWDGE queue: the sync engine's in-order stream is
            # saturated by phase A's eviction triggers and its queues only
            # complete these reads after all of phase A. Later pairs reuse
            # released phase-A SBUF, so they stay on the sync queue where
            # the WAR ordering is naturally enforced.
            eng = nc.scalar if pi < 2 else nc.sync
            qh = pool.tile([128, S], BF16, tag="qh", name=f"qh{pi}")
            eng.dma_start(qh[:], qT_ds[b][r0:r0 + 128, :])
            kh = pool.tile([128, S], BF16, tag="kh", name=f"kh{pi}")
            eng.dma_start(kh[:], kT_ds[b][r0:r0 + 128, :])
            vh = pool.tile([128, NKT, 128], BF16, tag="vh", name=f"vh{pi}")
            eng.dma_start(
                vh[:],
                v_ds[b][:, r0:r0 + 128].rearrange("(i p) d -> p i d", p=128),
            )
            pair_tiles[pi] = (qh, kh, vh)

        # ---------------- Phase A: QKV projection ----------------
        if "A" in phases:
         with tc.tile_pool(name="aw", bufs=1) as awp, \
             tc.tile_pool(name="ax", bufs=2) as axp, \
             tc.tile_pool(name="ast", bufs=6) as astp, \
             tc.tile_pool(name="aps", bufs=4, space="PSUM") as apsp:
            # wq + first x chunk first, split into 8-ktile chunks so the
            # first matmuls start after ~2MB of DMA instead of ~8.4MB.
            wq_sb = awp.tile([128, KT, COLS], BF16)
            wqr = wq.rearrange("(kt p) c -> p kt c", p=128)
            x0_sb = axp.tile([128, KT, 512], BF16, tag="x")
            x0r = xT[:, 0:512].rearrange("(kt p) t -> p kt t", p=128)
            chunks = [(0, 2), (2, 4)] + [(k, k + 4) for k in range(4, 32, 4)]
            for lo, hi in chunks:
                ks = slice(lo, hi)
                nc.sync.dma_start(wq_sb[:, ks, :], wqr[:, ks, :])
                nc.sync.dma_start(x0_sb[:, ks, :], x0r[:, ks, :])
            # wk in 8-ktile pieces: chunk 0's k groups run kt-outer and
            # consume these as they land (startup is DMA-bandwidth-paced)
            wk_sb = awp.tile([128, KT, COLS], BF16)
            wkr = wk.rearrange("(kt p) c -> p kt c", p=128)
            for k0 in range(0, KT, 8):
                nc.sync.dma_start(wk_sb[:, k0:k0 + 8, :], wkr[:, k0:k0 + 8, :])
            wv_sb = awp.tile([128, KT, COLS], BF16)
            nc.sync.dma_start(wv_sb[:], wv.rearrange("(kt p) c -> p kt c", p=128))

            for tck in range(TOK // 512):
                t0 = tck * 512
                if tck == 0:
                    x_sb = x0_sb
                else:
                    x_sb = axp.tile([128, KT, 512], BF16, tag="x")
                    nc.sync.dma_start(
                        x_sb[:],
                        xT[:, t0:t0 + 512].rearrange("(kt p) t -> p kt t", p=128),
                    )
                bb = tck // 4       # which batch these tokens belong to
                tl = t0 - bb * S    # token offset within the batch
                # qT / kT: [col-tile 128, tok 512], weight stationary.
                # Chunk 0 runs kt-outer (4 col groups per arriving DMA
                # chunk) so the DMA-paced startup never starves TensorE;
                # later chunks have DMA far ahead and run ct-outer.
                for half in range(2):
                    cts = range(half * 4, half * 4 + 4)
                    w_sb = wq_sb if half == 0 else wk_sb
                    b_sb = bq_sb if half == 0 else bk_sb
                    dst = qT_ds[bb] if half == 0 else kT_ds[bb]
                    pss = {}
                    for ct in cts:
                        pss[ct] = apsp.tile([128, 512], F32, tag="qk",
                                            name=f"apsqk{tck}_{ct}")
                    if tck == 0:
                        for kt in range(KT):
                            for ct in cts:
                                c0 = (ct % 4) * 128
                                nc.tensor.matmul(
                                    pss[ct][:], w_sb[:, kt, c0:c0 + 128],
                                    x_sb[:, kt, :],
                                    start=(kt == 0), stop=(kt == KT - 1),
                                )
                    else:
                        for ct in cts:
                            c0 = (ct % 4) * 128
                            for kt in range(KT):
                                nc.tensor.matmul(
                                    pss[ct][:], w_sb[:, kt, c0:c0 + 128],
                                    x_sb[:, kt, :],
                                    start=(kt == 0), stop=(kt == KT - 1),
                                )
                    for ct in cts:
                        c0 = (ct % 4) * 128
                        # eviction on DVE (idle in phase A) w/ fused bias add
                        st = astp.tile([128, 512], BF16, tag="qk_st")
                        nc.vector.tensor_scalar_add(
                            st[:], pss[ct][:], b_sb[:, ct % 4:ct % 4 + 1])
                        nc.sync.dma_start(dst[c0:c0 + 128, tl:tl + 512], st[:])
                # v: [tok-tile 128, col 512], x stationary
                for tt in range(4):
                    ps = apsp.tile([128, 512], F32, tag="v")
                    for kt in range(KT):
                        nc.tensor.matmul(
                            ps[:], x_sb[:, kt, tt * 128:(tt + 1) * 128],
                            wv_sb[:, kt, :],
                            start=(kt == 0), stop=False,
                        )
                    # bias: out[tok, col] += ones[tok] x b[col]
                    nc.tensor.matmul(
                        ps[:], ones_bf[:, 0:128], bv_sb[:],
                        start=False, stop=True,
                    )
                    st = astp.tile([128, 512], BF16, tag="v_st")
                    # ACT is idle all of phase A; splitting evictions
                    # ACT/DVE also drains the A tail faster (B's first
                    # score PSUM banks wait on A's last evictions)
                    nc.scalar.copy(st[:], ps[:])
                    nc.sync.dma_start(
                        v_ds[bb][tl + tt * 128:tl + (tt + 1) * 128, :], st[:])
                if tck == 3 and os.environ.get("PREFETCH", "1") == "1":
                    # batch-0 q/k/v fully staged: prefetch the first two
                    # head pairs now so phase B starts without a DMA wait.
                    ensure_pair_tiles(0, bqk0)
                    ensure_pair_tiles(1, bqk0)

        # ---------------- Phase B + C ----------------
        # ctxT survives phase B into phase C: [d, head, tok].
        ctxp = ctx.enter_context(tc.tile_pool(name="ctxp", bufs=1))
        ctxT = ctxp.tile([128, HL, TOK], BF16)

        # wo_sb's DMA is emitted from inside the B loop (see below): DMA
        # completion semaphores are shared across queues, so any matmul
        # emitted after a dma_start transitively waits on it -- emitting the
        # 4MB wo load at B's head would stall B's first PV group on it.
        cwp = ctx.enter_context(tc.tile_pool(name="cw", bufs=1))
        wo_sb = cwp.tile([128, HL, H], BF16)

        with tc.tile_pool(name="bqk", bufs=6) as bqkp, \
             tc.tile_pool(name="bpr", bufs=4) as bprp, \
             tc.tile_pool(name="bt8", bufs=2) as bt8p, \
             tc.tile_pool(name="bst", bufs=1) as bstp, \
             tc.tile_pool(name="bs2", bufs=2) as bst2p, \
             tc.tile_pool(name="bsc", bufs=2, space="PSUM") as bscp, \
             tc.tile_pool(name="bcx", bufs=2, space="PSUM") as bcxp, \
             tc.tile_pool(name="cst", bufs=4) as cstp, \
             tc.tile_pool(name="cps", bufs=2, space="PSUM") as cpsp:

            c_state = {"n": 0, "ready": 0, "caps": [(0, 0)]}
            PV_DEPTH = int(os.environ.get("PV_DEPTH", "2"))
            C_LAG = int(os.environ.get("C_LAG", "10"))
            TAIL2_LAG = int(os.environ.get("TAIL2_LAG", "3"))

            def emit_c_tile():
                # next output-projection tile, token-major within batch
                i = c_state["n"]
                c_state["n"] += 1
                ot, ncol = divmod(i, H // 512)
                t0 = ot * 128
                n0 = ncol * 512
                ps = cpsp.tile([128, 512], F32, tag="op", name=f"ops{i}")
                for hl in range(HL):
                    nc.tensor.matmul(
                        ps[:], ctxT[:, hl, t0:t0 + 128],
                        wo_sb[:, hl, n0:n0 + 512],
                        start=(hl == 0), stop=(hl == HL - 1),
                    )
                st = cstp.tile([128, 512], F32, tag="ost", name=f"ost{i}")
                if i % 2 == 0:
                    nc.scalar.copy(st[:], ps[:])
                else:
                    nc.vector.tensor_copy(st[:], ps[:])
                nc.sync.dma_start(out[t0:t0 + 128, n0:n0 + 512], st[:])

            # flat kj-unit schedule, qc-major within each batch: all four
            # heads finish a given qc block together, so output-projection
            # tiles for those tokens become C filler one qc later -- during
            # most of batch 0's otherwise ACT-bound window.
            units = [(b * HL + hl, qc, kj)
                     for b in range(NB)
                     for qc in range(S // 512)
                     for hl in range(HL)
                     for kj in range(NKT // 2)]

            state = {}   # per live qc: (cps, tmp8)
            pending = []  # units waiting for their PV emission (depth PV_DEPTH)

            def emit_pv(u):
                pi, qc, kj, probs_u = u
                _, _, vh = pair_tiles[pi]
                cps, _ = state[(pi, qc)]
                for ui in range(2):
                    ki = 2 * kj + ui
                    nc.tensor.matmul(
                        cps[:], vh[:, ki, :], probs_u[:, ui, :],
                        start=(ki == 0), stop=(ki == NKT - 1),
                    )

            deferred = []  # (emit_unit, lsb, dst, bump) awaiting recip+mul

            def emit_qc_tail(u, un):
                # ctx eviction (unnormalized) + softmax denominator adds +
                # gpsimd partition reduce. The reciprocal + normalize are
                # DEFERRED a few units (emit_qc_tail2): a reciprocal whose
                # gpsimd input isn't ready yet head-of-line blocks the DVE
                # FIFO for ~3.5us, which backlogs tmp8 adds/C evictions and
                # stalls the PE queue behind C-tile matmuls.
                pi, qc, kj, _ = u
                b, hl = divmod(pi, HL)
                s0 = b * S
                q0 = qc * 512
                cps, tmp8 = state.pop((pi, qc))
                dst = ctxT[:, hl, s0 + q0:s0 + q0 + 512]
                # evict on ACT: DVE's queue (pairwise adds + C evictions)
                # backlogs this copy and stalls the PV ring by ~1.3us per qc
                nc.scalar.copy(dst, cps[:])
                t4 = bstp.tile([128, 4, 512], BF16, tag="t4")
                nc.vector.tensor_add(
                    t4[:], tmp8[:, 0:8:2, :], tmp8[:, 1:8:2, :])
                t2 = bstp.tile([128, 2, 512], BF16, tag="t2")
                nc.vector.tensor_add(
                    t2[:], t4[:, 0:4:2, :], t4[:, 1:4:2, :])
                acc = bstp.tile([128, 512], F32, tag="acc")
                nc.vector.tensor_add(acc[:], t2[:, 0, :], t2[:, 1, :])
                lsb = bst2p.tile([128, 512], F32, tag="lsb",
                                 name=f"lsb{pi}_{qc}")
                nc.gpsimd.partition_all_reduce(
                    lsb[:], acc[:], channels=128,
                    reduce_op=bass_isa.ReduceOp.add)
                deferred.append((un, lsb, dst, pi % HL == HL - 1))

            MUL_ENGINE = os.environ.get("MUL_ENGINE", "gpsimd")

            def emit_qc_tail2(lsb, dst, bump, un):
                rec = bst2p.tile([128, 512], F32, tag="rec")
                nc.vector.reciprocal_approx_fast(out=rec[:], in_=lsb[:])
                # normalize on gpsimd: C-tile matmuls gate on this mul, and
                # the DVE queue backlogs ~15us at batch seams (4 qc chains
                # drain at once) — gpsimd is idle so the mul lands promptly
                if MUL_ENGINE == "gpsimd":
                    nc.gpsimd.tensor_mul(dst, dst, rec[:])
                else:
                    nc.vector.tensor_mul(dst, dst, rec[:])
                if bump:
                    # all 4 heads of this qc normalized (in emission order):
                    # its 32 C tiles become releasable C_LAG units later
                    c_state["ready"] += 32
                    c_state["caps"].append((un + C_LAG, c_state["ready"]))

            if "B" in phases:
                npairs = NB * HL
                for un, (pi, qc, kj) in enumerate(units):
                    b, hl = divmod(pi, HL)
                    s0 = b * S
                    q0 = qc * 512
                    if un == 0:
                        for pn in range(HL):
                            ensure_pair_tiles(pn, bqk0 if pn < 2 else bqkp)
                    elif un == 32:
                        # wo load is emitted here (not at B's head): DMA
                        # completion semaphores are shared, so everything
                        # emitted after it would transitively wait for it
                        nc.sync.dma_start(
                            wo_sb[:],
                            wo.rearrange("(hl p) n -> p hl n", p=128))
                    elif un == 64:
                        ensure_pair_tiles(HL, bqkp)
                        ensure_pair_tiles(HL + 1, bqkp)
                    elif un == 96:
                        ensure_pair_tiles(HL + 2, bqkp)
                        ensure_pair_tiles(HL + 3, bqkp)
                    qh, kh, _ = pair_tiles[pi]
                    if kj == 0:
                        cps = bcxp.tile([128, 512], F32, tag="ctx",
                                        name=f"cps{pi}_{qc}")
                        tmp8 = bt8p.tile([128, NKT // 2, 512], BF16,
                                         tag="tmp8", bufs=1,
                                         name=f"tmp8{pi}_{qc}")
                        state[(pi, qc)] = (cps, tmp8)
                    # scores for this unit: two 128-key tiles, one 2-bank
                    # PSUM tile so the exp runs 1024 wide
                    sps = bscp.tile([128, 2, 512], F32, tag="sc",
                                    name=f"sps{un}")
                    for ui in range(2):
                        ki = 2 * kj + ui
                        nc.tensor.matmul(
                            sps[:, ui, :],
                            kh[:, ki * 128:(ki + 1) * 128],
                            qh[:, q0:q0 + 512], start=True, stop=True,
                        )
                    # PV of an earlier unit lands here, after this unit's
                    # score matmuls. Depth-2 queue gives the ACT exp two
                    # units of TensorE work (~2.5us) of slack, so PV never
                    # blocks on a just-in-time exp completion.
                    while len(pending) >= PV_DEPTH:
                        u = pending.pop(0)
                        emit_pv(u)
                        if u[2] == NKT // 2 - 1:
                            emit_qc_tail(u, un)
                    # deferred recip+mul once the gpsimd reduce has had
                    # TAIL2_LAG units (~3us) to complete
                    while deferred and deferred[0][0] <= un - TAIL2_LAG:
                        _, lsb, dst, bump = deferred.pop(0)
                        emit_qc_tail2(lsb, dst, bump, un)
                    probs_u = bprp.tile([128, 2, 512], BF16, tag="probs",
                                        name=f"probs{un}")
                    nc.scalar.activation(probs_u[:], sps[:], AF.Exp,
                                         scale=SCALE)
                    _, tmp8 = state[(pi, qc)]
                    nc.vector.tensor_add(
                        tmp8[:, kj, :], probs_u[:, 0, :], probs_u[:, 1, :])
                    pending.append((pi, qc, kj, probs_u))
                    # C filler: one output tile per unit once its ctxT
                    # sources are done. During the last pair of each batch,
                    # token blocks of already-finished qc become available.
                    if "C" in phases:
                        # release at most one tile per unit, gated on the
                        # source qc's tails having been EMITTED (emission-
                        # order safety with the deeper PV queue) plus a
                        # C_LAG-unit cooldown for the normalize chain
                        cap = 0
                        for ua, cv in c_state["caps"]:
                            if un >= ua:
                                cap = max(cap, cv)
                        if c_state["n"] < cap:
                            emit_c_tile()
                for u in pending:
                    emit_pv(u)
                    if u[2] == NKT // 2 - 1:
                        emit_qc_tail(u, len(units))
                pending = []
                for ud, lsb, dst, bump in deferred:
                    emit_qc_tail2(lsb, dst, bump, len(units))
                deferred = []

            # ---------------- Phase C drain ----------------
            if "C" in phases:
                while c_state["n"] < TOK // 128 * (H // 512):
                    emit_c_tile()

    nc.compile()
    return nc


_NC = None


def _get_nc():
    global _NC
    if _NC is None:
        _NC = build_nc()
    return _NC


def _shard_inputs(x, attn_qkvw, attn_qkvb, attn_ow):
    bf = ml_dtypes.bfloat16
    x = np.asarray(x, dtype=np.float32)
    w = np.asarray(attn_qkvw, dtype=np.float32)
    b = np.asarray(attn_qkvb, dtype=np.float32)
    wo = np.asarray(attn_ow, dtype=np.float32)

    xT = np.ascontiguousarray(x.reshape(TOK, H).T).astype(bf)
    w4 = w.reshape(H, 3, 32, HD)
    b4 = b.reshape(3, 32, HD)
    in_maps = []
    for c in range(NCORES):
        hs = slice(c * HL, (c + 1) * HL)
        in_maps.append({
            "xT": xT,
            "wq": np.ascontiguousarray(w4[:, 0, hs, :].reshape(H, COLS)).astype(bf),
            "wk": np.ascontiguousarray(w4[:, 1, hs, :].reshape(H, COLS)).astype(bf),
            "wv": np.ascontiguousarray(w4[:, 2, hs, :].reshape(H, COLS)).astype(bf),
            "bq": np.ascontiguousarray(b4[0, hs, :].reshape(1, COLS)),
            "bk": np.ascontiguousarray(b4[1, hs, :].reshape(1, COLS)),
            "bv": b4[2, hs, :].reshape(1, COLS).astype(bf),
            "wo": np.ascontiguousarray(
                wo[c * COLS:(c + 1) * COLS, :]).astype(bf),
        })
    return in_maps


def kernel(x, attn_qkvw, attn_qkvb, attn_ow, attn_ob):
    import time as _time
    nc = _get_nc()
    in_maps = _shard_inputs(x, attn_qkvw, attn_qkvb, attn_ow)
    res = None
    for attempt in range(3):
        try:
            res = run_bass_kernel_spmd(nc, in_maps, core_ids=list(range(NCORES)))
            break
        except Exception:
            # transient NRT_EXEC_UNIT_UNRECOVERABLE has been observed on a
            # first dispatch; rebuild and retry once before giving up
            if attempt == 2:
                raise
            _time.sleep(2)
            global _NC
            _NC = None
            nc = _get_nc()
    acc = res.results[0]["out"]
    for c in range(1, NCORES):
        acc = acc + res.results[c]["out"]
    acc = acc + np.asarray(attn_ob, dtype=np.float32)[None, :]
    return acc.reshape(NB, S, H)

